# revision 23
# baseline (speedup 1.0000x reference)
"""Differential attention kernel for Trainium2, 8 NeuronCores.

Sharding: B(2) x head-groups(4) -> 8 cores; each core computes 3 heads'
differential attention for one batch element plus its partial slice of the
output projection (row-parallel over Wo). Host sums the 4 partials per batch
element and adds bo.

Per-core on-device pipeline (all matmuls bf16, fp32 PSUM accum):
  1. v-projection (natural [n, d] layout, +ones column for softmax denom)
  2. qT/kT projections (transposed layout [d, n]) for 6 units (3 heads x 2
     branches); score scale 1/sqrt(D) pre-folded into Wq on host
  3. per unit: S^T strips (keys on partitions) -> exp on ScalarE -> PT,
     PV matmuls accumulate u^T = P^T.T @ [v|1] into PSUM [q, 65]
     transposed out: u matmuls give [65, q]; row 64 = softmax denominator
  4. r = 1/denom (DMA-spread to [128, QH/128], full-lane reciprocal, gather
     back); R = broadcast via K=1 matmul (branch-2 broadcast folds in -lam);
     diff = u1*R1 + u2*R2 on VectorE
  5. output projection with per-head Wo row-slices -> partial [2048, 768]
"""

import os
import sys
from contextlib import ExitStack

for _p in ("/opt/trn_rl_repo", "/root/.axon_site/_ro/trn_rl_repo"):
    if os.path.isdir(_p) and _p not in sys.path:
        sys.path.insert(0, _p)

import ml_dtypes
import numpy as np

import concourse.bass as bass
import concourse.bacc as bacc_mod
import concourse.mybir as mybir
from concourse.bass_utils import run_bass_kernel_spmd
from concourse.tile import TileContext

BF16 = ml_dtypes.bfloat16
F = mybir.dt

B, N, C, H, D = 2, 2048, 768, 12, 64
HPC = 3          # heads per core
NCORES = 8
NT = N // 128    # 16 key strips / row tiles


def _body(nc, tc, ctx, xt, wqk, wv, wo, lamc, out, taps=None):
    fp32, bf16 = F.float32, F.bfloat16
    Exp = mybir.ActivationFunctionType.Exp

    singles = ctx.enter_context(tc.tile_pool(name="singles", bufs=1))
    wo_sb = singles.tile([64, HPC, C], bf16)       # per-head Wo rows
    lams_sb = singles.tile([128, 6], fp32)         # col u: 1.0 (br0) or -lam_h (br1)
    u_sb = singles.tile([64, 6, N], bf16)          # unnormalized PV outputs
    diff_sb = singles.tile([64, HPC, N], bf16)
    # DRAM bounce for r: DMA-broadcast across partitions needs a DRAM source
    r_dram = nc.dram_tensor("r_bounce", [6, N], bf16)

    nc.sync.dma_start(out=wo_sb, in_=wo[:, :].rearrange("(h p) c -> p h c", p=64))
    nc.sync.dma_start(out=lams_sb, in_=lamc[:, :])

    with tc.tile_pool(name="attn_sb", bufs=1) as attn_sb:
        qkv_sb = attn_sb.tile([128, 6, N], bf16)   # qT rows 0:64, kT rows 64:128
        k0_sb = attn_sb.tile([64, 6, N], bf16)     # kT re-homed to partition base 0
        v_sb = attn_sb.tile([128, NT, HPC, D + 1], bf16)
        nc.vector.memset(v_sb[:, :, :, D : D + 1], 1.0)

        # ---------- projections ----------
        with tc.tile_pool(name="proj_sb", bufs=1) as proj_sb:
            xt_sb = proj_sb.tile([128, 6, N], bf16)      # x^T, c = ch*128+p
            wqk_sb = proj_sb.tile([128, 6, 768], bf16)   # unit u at cols u*128..+128
            wv_sb = proj_sb.tile([128, 6, HPC * D], bf16)
            xt_r = xt[:, :].rearrange("(ch p) n -> p ch n", p=128)
            wqk_r = wqk[:, :].rearrange("(ch p) w -> p ch w", p=128)
            wv_r = wv[:, :].rearrange("(ch p) w -> p ch w", p=128)
            for c in range(6):
                nc.sync.dma_start(out=wv_sb[:, c, :], in_=wv_r[:, c, :])
            for c in range(6):
                nc.sync.dma_start(out=xt_sb[:, c, :], in_=xt_r[:, c, :])
            for c in range(6):
                nc.sync.dma_start(out=wqk_sb[:, c, :], in_=wqk_r[:, c, :])

            # v[n, d] for 3 heads, accumulated over 6 C-chunks
            with tc.tile_pool(name="vpp", bufs=3, space="PSUM") as vpp:
                for ti in range(NT):
                    vp = vpp.tile([128, HPC * D], fp32)
                    for c in range(6):
                        nc.tensor.matmul(
                            vp,
                            lhsT=xt_sb[:, c, ti * 128 : (ti + 1) * 128],
                            rhs=wv_sb[:, c, :],
                            start=(c == 0),
                            stop=(c == 5),
                        )
                    nc.vector.tensor_copy(
                        v_sb[:, ti, :, 0:D], vp.rearrange("p (h d) -> p h d", h=HPC)
                    )

            # qT/kT per unit (projT layout: proj columns on partitions)
            with tc.tile_pool(name="qpp", bufs=2, space="PSUM") as qpp:
                for u in range(6):
                    pp = qpp.tile([128, N], fp32)
                    for c in range(6):
                        for g in range(4):
                            nc.tensor.matmul(
                                pp[:, g * 512 : (g + 1) * 512],
                                lhsT=wqk_sb[:, c, u * 128 : (u + 1) * 128],
                                rhs=xt_sb[:, c, g * 512 : (g + 1) * 512],
                                start=(c == 0),
                                stop=(c == 5),
                            )
                    nc.vector.tensor_copy(qkv_sb[:, u, :], pp)
                    nc.sync.dma_start(out=k0_sb[:, u, :], in_=qkv_sb[64:128, u, :])
            if taps:
                nc.sync.dma_start(out=taps["qkv"][:, :, :], in_=qkv_sb)
                nc.sync.dma_start(out=taps["v"][:, :, :, :], in_=v_sb)

        # ---------- attention ----------
        # Process the two branch-units of each head as interleaved chains so
        # the PE always has independent work while ACT runs the other chain's
        # exp. Per-chain single-buffered PSUM, q processed in halves of 1024:
        # 2 (stA) + 2 (stB) + 2 (uA) + 2 (uB) = 8 banks.
        QH = 1024
        with tc.tile_pool(name="stp", bufs=1, space="PSUM") as stp, \
             tc.tile_pool(name="upp", bufs=1, space="PSUM") as upp, \
             tc.tile_pool(name="ptp", bufs=4) as ptp, \
             tc.tile_pool(name="rsc", bufs=2) as rsc:
            for h in range(HPC):
                for half in range(2):
                    q0 = half * QH
                    u_pair = []
                    for br in range(2):
                        u_ps = upp.tile([65, QH], fp32, tag=f"u{br}", name=f"u_ps{br}")
                        u_pair.append(u_ps)
                    for ti in range(NT):
                        for br in range(2):
                            u = 2 * h + br
                            st = stp.tile([128, QH], fp32, tag=f"st{br}", name=f"st{br}")
                            for g in range(2):
                                nc.tensor.matmul(
                                    st[:, g * 512 : (g + 1) * 512],
                                    lhsT=k0_sb[:, u, ti * 128 : (ti + 1) * 128],
                                    rhs=qkv_sb[0:64, u, q0 + g * 512 : q0 + (g + 1) * 512],
                                    start=True,
                                    stop=True,
                                )
                            pt = ptp.tile([128, QH], bf16)
                            nc.scalar.activation(pt, st, Exp)
                            for g in range(2):
                                nc.tensor.matmul(
                                    u_pair[br][:, g * 512 : (g + 1) * 512],
                                    lhsT=v_sb[:, ti, h, :],
                                    rhs=pt[:, g * 512 : (g + 1) * 512],
                                    start=(ti == 0),
                                    stop=(ti == NT - 1),
                                )
                    for br in range(2):
                        u = 2 * h + br
                        u_ps = u_pair[br]
                        # denominator row (psum partition 64) -> SBUF ->
                        # DMA-spread to [128, QH/128] at base partition 0 ->
                        # full-lane reciprocal -> DMA-gather back to the [1, N]
                        # row the broadcast matmul reads
                        dsc = rsc.tile([65, QH], fp32)
                        nc.vector.tensor_copy(dsc[64:65, :], u_ps[64:65, :])
                        den128 = rsc.tile([128, QH // 128], fp32)
                        nc.sync.dma_start(out=den128, in_=dsc[64:65, :])
                        r128 = rsc.tile([128, QH // 128], fp32)
                        nc.vector.reciprocal(r128, den128)
                        r128b = rsc.tile([128, QH // 128], bf16)
                        nc.vector.tensor_scalar_mul(r128b, r128, lams_sb[:, u : u + 1])
                        nc.sync.dma_start(out=r_dram[u : u + 1, q0 : q0 + QH], in_=r128b)
                        nc.vector.tensor_copy(u_sb[:, u, q0 : q0 + QH], u_ps[0:64, :])
                    # branch diff during attention: broadcast r across 64
                    # partitions via stride-0 DMA, then u1*R1 + u2*R2 on DVE
                    # (-lam_h already folded into branch-1's r)
                    rb0 = rsc.tile([64, QH], bf16, tag="rb0", name="rb0")
                    nc.sync.dma_start(
                        out=rb0,
                        in_=r_dram[2 * h : 2 * h + 1, q0 : q0 + QH].partition_broadcast(64),
                    )
                    rb1 = rsc.tile([64, QH], bf16, tag="rb1", name="rb1")
                    nc.sync.dma_start(
                        out=rb1,
                        in_=r_dram[2 * h + 1 : 2 * h + 2, q0 : q0 + QH].partition_broadcast(64),
                    )
                    t1 = rsc.tile([64, QH], bf16, tag="t1", name="t1")
                    nc.vector.tensor_mul(t1, u_sb[:, 2 * h, q0 : q0 + QH], rb0)
                    t2 = rsc.tile([64, QH], bf16, tag="t2", name="t2")
                    nc.vector.tensor_mul(t2, u_sb[:, 2 * h + 1, q0 : q0 + QH], rb1)
                    nc.vector.tensor_add(diff_sb[:, h, q0 : q0 + QH], t1, t2)

    if taps:
        nc.sync.dma_start(out=taps["u"][:, :, :], in_=u_sb)
        nc.sync.dma_start(out=taps["r"][0, :, :], in_=r_dram[:, :])

    # ---------- normalize, branch-diff, output projection ----------
    with tc.tile_pool(name="fpp", bufs=3, space="PSUM") as fpp, \
         tc.tile_pool(name="outp", bufs=3) as outp:
        for ti in range(NT):
            fo = fpp.tile([128, C], fp32)
            for hh in range(HPC):
                for o, w in ((0, 512), (512, 256)):
                    nc.tensor.matmul(
                        fo[:, o : o + w],
                        lhsT=diff_sb[:, hh, ti * 128 : (ti + 1) * 128],
                        rhs=wo_sb[:, hh, o : o + w],
                        start=(hh == 0),
                        stop=(hh == HPC - 1),
                    )
            ot = outp.tile([128, C], fp32)
            nc.vector.tensor_copy(ot, fo)
            nc.sync.dma_start(out=out[ti * 128 : (ti + 1) * 128, :], in_=ot)
        if taps:
            nc.sync.dma_start(out=taps["diff"][:, :, :], in_=diff_sb)


def build_bass(debug_taps=False):
    nc = bacc_mod.Bacc(None)
    xt = nc.dram_tensor("xt", [C, N], F.bfloat16, kind="ExternalInput")
    wqk = nc.dram_tensor("wqk", [C, 768], F.bfloat16, kind="ExternalInput")
    wv = nc.dram_tensor("wv", [C, HPC * D], F.bfloat16, kind="ExternalInput")
    wo = nc.dram_tensor("wo", [HPC * D, C], F.bfloat16, kind="ExternalInput")
    lamc = nc.dram_tensor("lamc", [128, 6], F.float32, kind="ExternalInput")
    out = nc.dram_tensor("out", [N, C], F.float32, kind="ExternalOutput")
    taps = None
    if debug_taps:
        taps = {
            "qkv": nc.dram_tensor("tap_qkv", [128, 6, N], F.bfloat16, kind="ExternalOutput"),
            "v": nc.dram_tensor("tap_v", [128, NT, HPC, D + 1], F.bfloat16, kind="ExternalOutput"),
            "u": nc.dram_tensor("tap_u", [64, 6, N], F.bfloat16, kind="ExternalOutput"),
            "r": nc.dram_tensor("tap_r", [1, 6, N], F.bfloat16, kind="ExternalOutput"),
            "diff": nc.dram_tensor("tap_diff", [64, HPC, N], F.bfloat16, kind="ExternalOutput"),
        }
    with TileContext(nc) as tc:
        with ExitStack() as ctx:
            _body(nc, tc, ctx, xt, wqk, wv, wo, lamc, out, taps=taps)
    nc.compile()
    return nc


_NC = None


def _get_nc():
    global _NC
    if _NC is None:
        _NC = build_bass()
    return _NC


def _prep_core(core, x, Wq, Wk, Wv, Wo, lam):
    b = core // 4
    heads = [(core % 4) * HPC + i for i in range(HPC)]
    sc = 1.0 / np.sqrt(D)
    xt = np.ascontiguousarray(x[b].T).astype(BF16)
    wqk = np.empty((C, 768), np.float32)
    for i, h in enumerate(heads):
        for br in range(2):
            u = 2 * i + br
            wqk[:, u * 128 : u * 128 + 64] = Wq[:, br * C + h * D : br * C + (h + 1) * D] * sc
            wqk[:, u * 128 + 64 : (u + 1) * 128] = Wk[:, br * C + h * D : br * C + (h + 1) * D]
    wv = np.concatenate([Wv[:, h * D : (h + 1) * D] for h in heads], axis=1)
    wo = np.concatenate([Wo[h * D : (h + 1) * D, :] for h in heads], axis=0)
    lams = np.zeros((128, 6), np.float32)
    for i, h in enumerate(heads):
        lams[:, 2 * i] = 1.0
        lams[:, 2 * i + 1] = -lam[h]
    return dict(
        xt=xt,
        wqk=wqk.astype(BF16),
        wv=wv.astype(BF16),
        wo=wo.astype(BF16),
        lamc=lams,
    )


def kernel(x, Wq, Wk, Wv, lambda_p, Wo, bo, _trace=False, _tmpdir=None):
    x = np.asarray(x, np.float32)
    lam = np.exp(np.asarray(lambda_p, np.float32).reshape(H))
    in_maps = [
        _prep_core(core, x, np.asarray(Wq, np.float32), np.asarray(Wk, np.float32),
                   np.asarray(Wv, np.float32), np.asarray(Wo, np.float32), lam)
        for core in range(NCORES)
    ]
    nc = _get_nc()
    res = run_bass_kernel_spmd(
        nc, in_maps, list(range(NCORES)), trace=_trace, tmpdir=_tmpdir
    )
    outf = np.zeros((B, N, C), np.float32)
    for core in range(NCORES):
        outf[core // 4] += res.results[core]["out"]
    outf += np.asarray(bo, np.float32)[None, None, :]
    if _trace:
        kernel.last_exec_time_ns = res.exec_time_ns
    return outf


# revision 26
# speedup vs baseline: 1.0537x; 1.0537x over previous
"""Differential attention kernel for Trainium2, 8 NeuronCores.

Sharding: B(2) x head-groups(4) -> 8 cores; each core computes 3 heads'
differential attention for one batch element plus its partial slice of the
output projection (row-parallel over Wo). Host sums the 4 partials per batch
element and adds bo.

Per-core on-device pipeline (all matmuls bf16, fp32 PSUM accum):
  1. v-projection (natural [n, d] layout, +ones column for softmax denom)
  2. qT/kT projections (transposed layout [d, n]) for 6 units (3 heads x 2
     branches); score scale 1/sqrt(D) pre-folded into Wq on host
  3. per unit: S^T strips (keys on partitions) -> exp on ScalarE -> PT,
     PV matmuls accumulate u^T = P^T.T @ [v|1] into PSUM [q, 65]
     transposed out: u matmuls give [65, q]; row 64 = softmax denominator
  4. r = 1/denom (DMA-spread to [128, QH/128], full-lane reciprocal, gather
     back); R = broadcast via K=1 matmul (branch-2 broadcast folds in -lam);
     diff = u1*R1 + u2*R2 on VectorE
  5. output projection with per-head Wo row-slices -> partial [2048, 768]
"""

import os
import sys
from contextlib import ExitStack

for _p in ("/opt/trn_rl_repo", "/root/.axon_site/_ro/trn_rl_repo"):
    if os.path.isdir(_p) and _p not in sys.path:
        sys.path.insert(0, _p)

import ml_dtypes
import numpy as np

import concourse.bass as bass
import concourse.bacc as bacc_mod
import concourse.mybir as mybir
from concourse.bass_utils import run_bass_kernel_spmd
from concourse.tile import TileContext

BF16 = ml_dtypes.bfloat16
F = mybir.dt

B, N, C, H, D = 2, 2048, 768, 12, 64
HPC = 3          # heads per core
NCORES = 8
NT = N // 128    # 16 key strips / row tiles


def _body(nc, tc, ctx, xt, wqk, wv, wo, lamc, out, taps=None):
    fp32, bf16 = F.float32, F.bfloat16
    Exp = mybir.ActivationFunctionType.Exp

    singles = ctx.enter_context(tc.tile_pool(name="singles", bufs=1))
    wo_sb = singles.tile([64, HPC, C], bf16)       # per-head Wo rows
    lams_sb = singles.tile([128, 6], fp32)         # col u: 1.0 (br0) or -lam_h (br1)
    u_sb = singles.tile([64, 6, N], bf16)          # unnormalized PV outputs
    diff_sb = singles.tile([64, HPC, N], bf16)
    # DRAM bounce for r: DMA-broadcast across partitions needs a DRAM source
    r_dram = nc.dram_tensor("r_bounce", [6, N], bf16)

    nc.sync.dma_start(out=wo_sb, in_=wo[:, :].rearrange("(h p) c -> p h c", p=64))
    nc.sync.dma_start(out=lams_sb, in_=lamc[:, :])

    with tc.tile_pool(name="attn_sb", bufs=1) as attn_sb:
        qkv_sb = attn_sb.tile([128, 6, N], bf16)   # qT rows 0:64, kT rows 64:128
        k0_sb = attn_sb.tile([64, 6, N], bf16)     # kT re-homed to partition base 0
        v_sb = attn_sb.tile([128, NT, HPC, D + 1], bf16)
        nc.vector.memset(v_sb[:, :, :, D : D + 1], 1.0)

        # ---------- projections ----------
        with tc.tile_pool(name="proj_sb", bufs=1) as proj_sb:
            xt_sb = proj_sb.tile([128, 6, N], bf16)      # x^T, c = ch*128+p
            wqk_sb = proj_sb.tile([128, 6, 768], bf16)   # unit u at cols u*128..+128
            wv_sb = proj_sb.tile([128, 6, HPC * D], bf16)
            xt_r = xt[:, :].rearrange("(ch p) n -> p ch n", p=128)
            wqk_r = wqk[:, :].rearrange("(ch p) w -> p ch w", p=128)
            wv_r = wv[:, :].rearrange("(ch p) w -> p ch w", p=128)
            for c in range(6):
                nc.sync.dma_start(out=wv_sb[:, c, :], in_=wv_r[:, c, :])
            for c in range(6):
                eng = nc.sync if c % 2 == 0 else nc.gpsimd
                eng.dma_start(out=xt_sb[:, c, :], in_=xt_r[:, c, :])
            for c in range(6):
                nc.sync.dma_start(out=wqk_sb[:, c, :], in_=wqk_r[:, c, :])

            # v[n, d] for 3 heads, accumulated over 6 C-chunks
            with tc.tile_pool(name="vpp", bufs=3, space="PSUM") as vpp:
                for ti in range(NT):
                    vp = vpp.tile([128, HPC * D], fp32)
                    for c in range(6):
                        nc.tensor.matmul(
                            vp,
                            lhsT=xt_sb[:, c, ti * 128 : (ti + 1) * 128],
                            rhs=wv_sb[:, c, :],
                            start=(c == 0),
                            stop=(c == 5),
                        )
                    nc.vector.tensor_copy(
                        v_sb[:, ti, :, 0:D], vp.rearrange("p (h d) -> p h d", h=HPC)
                    )

            # qT/kT per unit (projT layout: proj columns on partitions)
            with tc.tile_pool(name="qpp", bufs=2, space="PSUM") as qpp:
                for u in range(6):
                    pp = qpp.tile([128, N], fp32)
                    for c in range(6):
                        for g in range(4):
                            nc.tensor.matmul(
                                pp[:, g * 512 : (g + 1) * 512],
                                lhsT=wqk_sb[:, c, u * 128 : (u + 1) * 128],
                                rhs=xt_sb[:, c, g * 512 : (g + 1) * 512],
                                start=(c == 0),
                                stop=(c == 5),
                            )
                    nc.vector.tensor_copy(qkv_sb[:, u, :], pp)
                    nc.sync.dma_start(out=k0_sb[:, u, :], in_=qkv_sb[64:128, u, :])
            if taps:
                nc.sync.dma_start(out=taps["qkv"][:, :, :], in_=qkv_sb)
                nc.sync.dma_start(out=taps["v"][:, :, :, :], in_=v_sb)

        # ---------- attention ----------
        # Process the two branch-units of each head as interleaved chains so
        # the PE always has independent work while ACT runs the other chain's
        # exp. Per-chain single-buffered PSUM, q processed in halves of 1024:
        # 2 (stA) + 2 (stB) + 2 (uA) + 2 (uB) = 8 banks.
        QH = 1024
        with tc.tile_pool(name="stp", bufs=1, space="PSUM") as stp, \
             tc.tile_pool(name="upp", bufs=1, space="PSUM") as upp, \
             tc.tile_pool(name="ptp", bufs=4) as ptp, \
             tc.tile_pool(name="rsc", bufs=2) as rsc:
            for h in range(HPC):
                for half in range(2):
                    q0 = half * QH
                    u_pair = []
                    for br in range(2):
                        u_ps = upp.tile([65, QH], fp32, tag=f"u{br}", name=f"u_ps{br}")
                        u_pair.append(u_ps)
                    # software pipeline: PV runs one strip behind S^T/exp so
                    # the PE always has PV work that is not gated by the
                    # current strip's exp
                    pt_prev = [None, None]
                    for ti in range(NT + 1):
                        pt_cur = [None, None]
                        if ti < NT:
                            for br in range(2):
                                u = 2 * h + br
                                st = stp.tile([128, QH], fp32, tag=f"st{br}", name=f"st{br}")
                                for g in range(2):
                                    nc.tensor.matmul(
                                        st[:, g * 512 : (g + 1) * 512],
                                        lhsT=k0_sb[:, u, ti * 128 : (ti + 1) * 128],
                                        rhs=qkv_sb[0:64, u, q0 + g * 512 : q0 + (g + 1) * 512],
                                        start=True,
                                        stop=True,
                                    )
                                pt = ptp.tile([128, QH], bf16, tag=f"pt{br}", name=f"pt{br}")
                                nc.scalar.activation(pt, st, Exp)
                                pt_cur[br] = pt
                        if ti > 0:
                            tprev = ti - 1
                            for br in range(2):
                                for g in range(2):
                                    nc.tensor.matmul(
                                        u_pair[br][:, g * 512 : (g + 1) * 512],
                                        lhsT=v_sb[:, tprev, h, :],
                                        rhs=pt_prev[br][:, g * 512 : (g + 1) * 512],
                                        start=(tprev == 0),
                                        stop=(tprev == NT - 1),
                                    )
                        pt_prev = pt_cur
                    for br in range(2):
                        u = 2 * h + br
                        u_ps = u_pair[br]
                        # denominator row (psum partition 64) -> SBUF ->
                        # DMA-spread to [128, QH/128] at base partition 0 ->
                        # full-lane reciprocal -> DMA-gather back to the [1, N]
                        # row the broadcast matmul reads
                        dsc = rsc.tile([65, QH], fp32)
                        nc.vector.tensor_copy(dsc[64:65, :], u_ps[64:65, :])
                        den128 = rsc.tile([128, QH // 128], fp32)
                        nc.sync.dma_start(out=den128, in_=dsc[64:65, :])
                        r128 = rsc.tile([128, QH // 128], fp32)
                        nc.vector.reciprocal(r128, den128)
                        r128b = rsc.tile([128, QH // 128], bf16)
                        nc.vector.tensor_scalar_mul(r128b, r128, lams_sb[:, u : u + 1])
                        nc.sync.dma_start(out=r_dram[u : u + 1, q0 : q0 + QH], in_=r128b)
                        nc.vector.tensor_copy(u_sb[:, u, q0 : q0 + QH], u_ps[0:64, :])
                    # branch diff during attention: broadcast r across 64
                    # partitions via stride-0 DMA, then u1*R1 + u2*R2 on DVE
                    # (-lam_h already folded into branch-1's r)
                    rb0 = rsc.tile([64, QH], bf16, tag="rb0", name="rb0")
                    nc.sync.dma_start(
                        out=rb0,
                        in_=r_dram[2 * h : 2 * h + 1, q0 : q0 + QH].partition_broadcast(64),
                    )
                    rb1 = rsc.tile([64, QH], bf16, tag="rb1", name="rb1")
                    nc.sync.dma_start(
                        out=rb1,
                        in_=r_dram[2 * h + 1 : 2 * h + 2, q0 : q0 + QH].partition_broadcast(64),
                    )
                    t1 = rsc.tile([64, QH], bf16, tag="t1", name="t1")
                    nc.vector.tensor_mul(t1, u_sb[:, 2 * h, q0 : q0 + QH], rb0)
                    t2 = rsc.tile([64, QH], bf16, tag="t2", name="t2")
                    nc.vector.tensor_mul(t2, u_sb[:, 2 * h + 1, q0 : q0 + QH], rb1)
                    nc.vector.tensor_add(diff_sb[:, h, q0 : q0 + QH], t1, t2)

    if taps:
        nc.sync.dma_start(out=taps["u"][:, :, :], in_=u_sb)
        nc.sync.dma_start(out=taps["r"][0, :, :], in_=r_dram[:, :])

    # ---------- normalize, branch-diff, output projection ----------
    with tc.tile_pool(name="fpp", bufs=3, space="PSUM") as fpp, \
         tc.tile_pool(name="outp", bufs=3) as outp:
        for ti in range(NT):
            fo = fpp.tile([128, C], fp32)
            for hh in range(HPC):
                for o, w in ((0, 512), (512, 256)):
                    nc.tensor.matmul(
                        fo[:, o : o + w],
                        lhsT=diff_sb[:, hh, ti * 128 : (ti + 1) * 128],
                        rhs=wo_sb[:, hh, o : o + w],
                        start=(hh == 0),
                        stop=(hh == HPC - 1),
                    )
            ot = outp.tile([128, C], fp32)
            nc.vector.tensor_copy(ot, fo)
            oeng = nc.sync if ti % 2 == 0 else nc.gpsimd
            oeng.dma_start(out=out[ti * 128 : (ti + 1) * 128, :], in_=ot)
        if taps:
            nc.sync.dma_start(out=taps["diff"][:, :, :], in_=diff_sb)


def build_bass(debug_taps=False):
    nc = bacc_mod.Bacc(None)
    xt = nc.dram_tensor("xt", [C, N], F.bfloat16, kind="ExternalInput")
    wqk = nc.dram_tensor("wqk", [C, 768], F.bfloat16, kind="ExternalInput")
    wv = nc.dram_tensor("wv", [C, HPC * D], F.bfloat16, kind="ExternalInput")
    wo = nc.dram_tensor("wo", [HPC * D, C], F.bfloat16, kind="ExternalInput")
    lamc = nc.dram_tensor("lamc", [128, 6], F.float32, kind="ExternalInput")
    out = nc.dram_tensor("out", [N, C], F.float32, kind="ExternalOutput")
    taps = None
    if debug_taps:
        taps = {
            "qkv": nc.dram_tensor("tap_qkv", [128, 6, N], F.bfloat16, kind="ExternalOutput"),
            "v": nc.dram_tensor("tap_v", [128, NT, HPC, D + 1], F.bfloat16, kind="ExternalOutput"),
            "u": nc.dram_tensor("tap_u", [64, 6, N], F.bfloat16, kind="ExternalOutput"),
            "r": nc.dram_tensor("tap_r", [1, 6, N], F.bfloat16, kind="ExternalOutput"),
            "diff": nc.dram_tensor("tap_diff", [64, HPC, N], F.bfloat16, kind="ExternalOutput"),
        }
    with TileContext(nc) as tc:
        with ExitStack() as ctx:
            _body(nc, tc, ctx, xt, wqk, wv, wo, lamc, out, taps=taps)
    nc.compile()
    return nc


_NC = None


def _get_nc():
    global _NC
    if _NC is None:
        _NC = build_bass()
    return _NC


def _prep_core(core, x, Wq, Wk, Wv, Wo, lam):
    b = core // 4
    heads = [(core % 4) * HPC + i for i in range(HPC)]
    sc = 1.0 / np.sqrt(D)
    xt = np.ascontiguousarray(x[b].T).astype(BF16)
    wqk = np.empty((C, 768), np.float32)
    for i, h in enumerate(heads):
        for br in range(2):
            u = 2 * i + br
            wqk[:, u * 128 : u * 128 + 64] = Wq[:, br * C + h * D : br * C + (h + 1) * D] * sc
            wqk[:, u * 128 + 64 : (u + 1) * 128] = Wk[:, br * C + h * D : br * C + (h + 1) * D]
    wv = np.concatenate([Wv[:, h * D : (h + 1) * D] for h in heads], axis=1)
    wo = np.concatenate([Wo[h * D : (h + 1) * D, :] for h in heads], axis=0)
    lams = np.zeros((128, 6), np.float32)
    for i, h in enumerate(heads):
        lams[:, 2 * i] = 1.0
        lams[:, 2 * i + 1] = -lam[h]
    return dict(
        xt=xt,
        wqk=wqk.astype(BF16),
        wv=wv.astype(BF16),
        wo=wo.astype(BF16),
        lamc=lams,
    )


def kernel(x, Wq, Wk, Wv, lambda_p, Wo, bo, _trace=False, _tmpdir=None):
    x = np.asarray(x, np.float32)
    lam = np.exp(np.asarray(lambda_p, np.float32).reshape(H))
    in_maps = [
        _prep_core(core, x, np.asarray(Wq, np.float32), np.asarray(Wk, np.float32),
                   np.asarray(Wv, np.float32), np.asarray(Wo, np.float32), lam)
        for core in range(NCORES)
    ]
    nc = _get_nc()
    res = run_bass_kernel_spmd(
        nc, in_maps, list(range(NCORES)), trace=_trace, tmpdir=_tmpdir
    )
    outf = np.zeros((B, N, C), np.float32)
    for core in range(NCORES):
        outf[core // 4] += res.results[core]["out"]
    outf += np.asarray(bo, np.float32)[None, None, :]
    if _trace:
        kernel.last_exec_time_ns = res.exec_time_ns
    return outf
